# revision 1
# baseline (speedup 1.0000x reference)
"""Bass/Tile TRN2 kernel for nn_Disen_GAT_For_Multi_Aspect.

Contract: kernel(**inputs) takes FULL fp32 numpy inputs (keys as in
reference.setup_inputs()) and returns the FULL [B, A, H] fp32 output.

Strategy
--------
Data-parallel over batch B across the 8 cores (1 batch row / core, A=4
aspects per core).  Heavy algebraic restructuring of the reference:

  q   = Wq^T asp + bq                          (per aspect, [H])
  u   = transA q ; v = transB q ; y = W1b v ; a3 = W1a^T q
  QT  = einsum(q, T1)   -> w = QT^T v          (one batched T1 pass)
  G   = Wk @ {q,w,y,u}                         ([D, k] per aspect)

  logits collapse to matvec rows against the raw (transposed) inputs:
    ST = (Wk q).T_n,  SX = (Wk{q,w,y}).X_n,  SD = (Wk u).Dp_n
  combined with mask-power rows M^k/SCALE and scalar bias terms.

  Only V_W = Wv^T X^T and V_T = Wv^T T^T are materialized [H,N];
  att_z[h] = sum_n r[n] V_W[h,n] V_T[h,n],  r = att_row * M^3.

Big streams (X^T, T^T, Dp^T, T1) ship as bf16; fp32 PSUM accumulate.
Phasing: V/ST/SD streams for aspects 0-1 run before the T1 pass (they
don't depend on it) to keep the PE warm; row algebra is aspect-stacked
into [4, 512] tiles so DVE ops use 4 lanes and 1/4 the instructions.
Constants ship as two packed tensors (one f32, one bf16) = 2 DMAs, and
DMA issue is spread across the sync/vector/scalar/gpsimd sequencers.
"""

import contextlib
import ctypes
import sys
import types

import numpy as np
import ml_dtypes

import concourse.bacc as bacc
import concourse.mybir as mybir
import concourse.tile as tile
from concourse.bass_utils import run_bass_kernel_spmd

B, A, N, D, H = 8, 4, 512, 1024, 128
SCALE = float(np.sqrt(H))
NCORES = 8
DC = D // H  # 8 contraction chunks of 128

F32 = mybir.dt.float32
BF16 = mybir.dt.bfloat16
BF = ml_dtypes.bfloat16
AF = mybir.ActivationFunctionType
OP = mybir.AluOpType

# cpack (f32) column layout
C_WKT = 0              # [128, 1024]  Wk^T
C_WQ = 1024            # [128, 8, 128] Wq chunk-packed
C_TAT = 2048           # transA^T
C_TBT = 2176           # transB^T
C_W1A = 2304           # W1a (raw)
C_W1BT = 2432          # W1b^T
C_BIAS = 2560          # [bq|bk|bv|b1|tb]
C_ASP = 2565           # [128, 8, 4] aspect^T chunk-packed
C_MASK = 2597          # rows 0-3: fmask replicated [4, 512]
C_COMBW = 3109         # rows 0-3: comb_w replicated [4, 3]
C_W = 3112
# bpack (bf16) column layout
B_WV = 0               # [128, 8, 128] Wv chunk-packed
B_E4 = 1024            # rows 0-3: E4[k, a*128+p] = (k == a)
B_W = 1536

LAST_RESULTS = None  # test harness peeks at this


def _build():
    nc = bacc.Bacc("TRN2", target_bir_lowering=False, debug=False,
                   num_devices=NCORES)

    xtd = nc.dram_tensor("xtd", [A, 128, DC, 3, N], BF16,
                         kind="ExternalInput")
    t1f = nc.dram_tensor("t1f", [H, H * H], mybir.dt.float8e4, kind="ExternalInput")
    cpackd = nc.dram_tensor("cpack", [128, C_W], F32, kind="ExternalInput")
    cpackh = nc.dram_tensor("cpackh", [128, 1057], F32, kind="ExternalInput")
    bpackd = nc.dram_tensor("bpack", [128, B_W], BF16, kind="ExternalInput")
    out = nc.dram_tensor("out", [H, A], F32, kind="ExternalOutput")
    qtf_dram = nc.dram_tensor("qtf_dram", [A, H * H], BF16)

    with tile.TileContext(nc) as tc:
        with (
            tc.tile_pool(name="const", bufs=1) as cp,
            tc.tile_pool(name="stream", bufs=2) as sp,
            tc.tile_pool(name="vwt", bufs=2) as vp,
            tc.tile_pool(name="rows", bufs=2) as rp,
            tc.tile_pool(name="big", bufs=2) as bp,
            tc.tile_pool(name="t1p", bufs=4) as t1p,
            tc.tile_pool(name="vps", bufs=4, space="PSUM") as vps,
            tc.tile_pool(name="sxps", bufs=3, space="PSUM") as rps,
            tc.tile_pool(name="smallps", bufs=1, space="PSUM") as sps,
        ):
            # ---- packed constants (2 DMAs) + T1 ----------------------
            cph = cp.tile([128, 1057], F32, tag="cph")
            nc.sync.dma_start(out=cph, in_=cpackh.ap())
            cpk = cp.tile([128, C_W], F32, tag="cpk")
            nc.sync.dma_start(out=cpk, in_=cpackd.ap())
            bpk = cp.tile([128, B_W], BF16, tag="bpk")
            nc.sync.dma_start(out=bpk, in_=bpackd.ap())

            wkt_sb = cpk[:, C_WKT:C_WKT + D]
            wq_v = cph[:, 0:DC * H].rearrange("p (c h) -> p c h", c=DC)
            tat_sb = cpk[:, C_TAT:C_TAT + H]
            tbt_sb = cpk[:, C_TBT:C_TBT + H]
            w1a_sb = cpk[:, C_W1A:C_W1A + H]
            w1bt_sb = cpk[:, C_W1BT:C_W1BT + H]
            bq_c = cph[:, 1056:1057]
            bk_c = cpk[:, C_BIAS + 1:C_BIAS + 2]
            bv_c = cpk[:, C_BIAS + 2:C_BIAS + 3]
            b1_c = cpk[:, C_BIAS + 3:C_BIAS + 4]
            tb_c = cpk[:, C_BIAS + 4:C_BIAS + 5]
            asp_v = cph[:, DC * H:DC * H + DC * A].rearrange(
                "p (c a) -> p c a", c=DC)
            mrep = cpk[0:4, C_MASK:C_MASK + N]       # [4, N]
            combw4 = cpk[0:4, C_COMBW:C_COMBW + 3]   # [4, 3]
            wv_v = bpk[:, B_WV:B_WV + DC * H].rearrange(
                "p (c h) -> p c h", c=DC)

            ones_col = cp.tile([128, 1], F32, tag="ones_col")
            nc.vector.memset(ones_col, 1.0)

            # ---- mask-power rows, all [4, N] -------------------------
            inv_s = 1.0 / SCALE
            neg_r = cp.tile([4, N], F32, tag="neg_r")
            nc.vector.tensor_scalar(neg_r, mrep, 1e30, 1e30,
                                    op0=OP.mult, op1=OP.subtract)

            # ---- small chain (T1-independent part) -------------------
            ps_q = sps.tile([H, A], F32, tag="sps")
            for c in range(DC):
                nc.tensor.matmul(ps_q, lhsT=wq_v[:, c, :],
                                 rhs=asp_v[:, c, :],
                                 start=(c == 0), stop=(c == DC - 1))
            q4 = cp.tile([H, A], F32, tag="q4")
            nc.scalar.activation(q4, ps_q, AF.Identity, bias=bq_c)
            q4bf = cp.tile([H, A], BF16, tag="q4bf")
            nc.vector.tensor_copy(q4bf, q4)
            q4f8 = cp.tile([H, A], mybir.dt.float8e4, tag="q4f8")
            nc.vector.tensor_copy(q4f8, q4)

            # qwyu: [q|w|y|u] x 4 aspects (w filled after the T1 pass)
            qwyu = cp.tile([H, 16], F32, tag="qwyu")
            nc.vector.tensor_copy(qwyu[:, 0:4], q4)

            ps_s = sps.tile([H, A], F32, tag="sps")
            nc.tensor.matmul(ps_s, lhsT=tbt_sb, rhs=q4, start=True, stop=True)
            v4 = cp.tile([H, A], F32, tag="v4")
            nc.vector.tensor_copy(v4, ps_s)
            v4bf = cp.tile([H, A], BF16, tag="v4bf")
            nc.vector.tensor_copy(v4bf, ps_s)

            ps_s = sps.tile([H, A], F32, tag="sps")
            nc.tensor.matmul(ps_s, lhsT=tat_sb, rhs=q4, start=True, stop=True)
            nc.vector.tensor_copy(qwyu[:, 12:16], ps_s)  # u

            ps_s = sps.tile([H, A], F32, tag="sps")
            nc.tensor.matmul(ps_s, lhsT=w1bt_sb, rhs=v4, start=True,
                             stop=True)
            nc.vector.tensor_copy(qwyu[:, 8:12], ps_s)  # y

            ps_s = sps.tile([H, A], F32, tag="sps")
            nc.tensor.matmul(ps_s, lhsT=w1a_sb, rhs=q4, start=True, stop=True)
            a3q = cp.tile([H, A], F32, tag="a3q")
            nc.vector.tensor_copy(a3q, ps_s)

            # gE = Wk @ [q|u]  (T1-independent; feeds ST and SD)
            qu8 = cp.tile([H, 8], F32, tag="qu8")
            nc.vector.tensor_copy(qu8[:, 0:4], q4)
            nc.vector.tensor_copy(qu8[:, 4:8], qwyu[:, 12:16])
            gE = cp.tile([128, DC, 8], BF16, tag="gE")
            for c in range(DC):
                ps_g = sps.tile([128, 8], F32, tag="sps")
                nc.tensor.matmul(ps_g, lhsT=wkt_sb[:, c * H:(c + 1) * H],
                                 rhs=qu8, start=True, stop=True)
                nc.scalar.copy(gE[:, c, :], ps_g)
            # gE[:, c, 0:4] = Wk q (ST), gE[:, c, 4:8] = Wk u (SD)
            gEv = gE[:, :, :].rearrange("p c (v a) -> p c v a", v=2)

            # ---- per-aspect stream machinery -------------------------
            va_tiles = {}
            rows5 = cp.tile([4, 5 * N], F32, tag="rows5")
            # layout per aspect row: [st | sd | sx0 | sx1 | sx2]

            def stream_vstsd(a, xa):
                # xa: [128, DC, 3, N] interleaved [X|T|Dp]
                ps_vw = vps.tile([H, N], F32, tag="vps")
                ps_vt = vps.tile([H, N], F32, tag="vps")
                ps_td = rps.tile([2, N], F32, tag="rps")
                ps_sd = rps.tile([2, N], F32, tag="rps")
                for c in range(DC):
                    nc.tensor.matmul(ps_vw, lhsT=wv_v[:, c, :],
                                     rhs=xa[:, c, 0, :], start=(c == 0),
                                     stop=(c == DC - 1))
                    nc.tensor.matmul(ps_vt, lhsT=wv_v[:, c, :],
                                     rhs=xa[:, c, 1, :], start=(c == 0),
                                     stop=(c == DC - 1))
                    nc.tensor.matmul(ps_td, lhsT=gEv[:, c, :, a],
                                     rhs=xa[:, c, 1, :], start=(c == 0),
                                     stop=(c == DC - 1))
                    nc.tensor.matmul(ps_sd, lhsT=gEv[:, c, :, a],
                                     rhs=xa[:, c, 2, :], start=(c == 0),
                                     stop=(c == DC - 1))
                vv_a = vp.tile([H, 2 * N], F32, tag="vwt")
                nc.scalar.activation(vv_a[:, 0:N], ps_vw, AF.Identity,
                                     bias=bv_c)
                nc.scalar.activation(vv_a[:, N:2 * N], ps_vt, AF.Identity,
                                     bias=bv_c)
                tdst = rp.tile([2, 2 * N], F32, tag="tdst", bufs=1)
                nc.vector.tensor_copy(tdst[:, 0:N], ps_td)
                nc.vector.tensor_copy(tdst[:, N:2 * N], ps_sd)
                nc.scalar.dma_start(out=rows5[a:a + 1, 0:N],
                                    in_=tdst[0:1, 0:N])
                nc.scalar.dma_start(out=rows5[a:a + 1, N:2 * N],
                                    in_=tdst[1:2, N:2 * N])
                va_tiles[a] = vv_a

            def stream_sx(a, xa, g4v):
                ps_sx = rps.tile([3, N], F32, tag="rps")
                for c in range(DC):
                    nc.tensor.matmul(ps_sx, lhsT=g4v[:, c, :, a],
                                     rhs=xa[:, c, 0, :], start=(c == 0),
                                     stop=(c == DC - 1))
                sxs = rp.tile([3, N], F32, tag="sxs")
                nc.vector.tensor_copy(sxs, ps_sx)
                for r in range(3):
                    nc.sync.dma_start(
                        out=rows5[a:a + 1, (2 + r) * N:(3 + r) * N],
                        in_=sxs[r:r + 1, :])

            # ---- T1 pass: QT = q . T1, then w = QT^T v ---------------
            qstg = None
            t1_sb = None
            for mcol in range(H * H // 512):
                if mcol % 8 == 0:
                    t1_sb = t1p.tile([128, 8 * 512], mybir.dt.float8e4,
                                     tag="t1")
                    pb = mcol * 512
                    nc.scalar.dma_start(out=t1_sb,
                                          in_=t1f.ap()[:, pb:pb + 8 * 512])
                ps_r = rps.tile([A, 512], F32, tag="rps")
                toff = (mcol % 8) * 512
                nc.tensor.matmul(ps_r, lhsT=q4f8,
                                 rhs=t1_sb[:, toff:toff + 512],
                                 start=True, stop=True)
                if mcol % 4 == 0:
                    qstg = rp.tile([A, 4 * 512], BF16, tag="qstg")
                off = (mcol % 4) * 512
                nc.vector.tensor_copy(qstg[:, off:off + 512], ps_r)
                if mcol % 4 == 3:
                    base = (mcol - 3) * 512
                    nc.scalar.dma_start(
                        out=qtf_dram.ap()[:, base:base + 4 * 512], in_=qstg)

            for a in range(A):
                qta = cp.tile([H, H], BF16, tag=f"qta{a}")
                nc.scalar.dma_start(out=qta, in_=qtf_dram.ap()[a].rearrange(
                    "(j k) -> j k", j=H))
                ps_w = sps.tile([H, 1], F32, tag="sps")
                nc.tensor.matmul(ps_w, lhsT=qta, rhs=v4bf[:, a:a + 1],
                                 start=True, stop=True)
                nc.vector.tensor_copy(qwyu[:, 4 + a:5 + a], ps_w)  # w

            # g4 = Wk @ [q|w|y] for SX
            g4 = cp.tile([128, DC, 12], BF16, tag="g4")
            for c in range(DC):
                ps_g = sps.tile([128, 12], F32, tag="sps")
                nc.tensor.matmul(ps_g, lhsT=wkt_sb[:, c * H:(c + 1) * H],
                                 rhs=qwyu[:, 0:12], start=True, stop=True)
                nc.scalar.copy(g4[:, c, :], ps_g)
            g4v = g4[:, :, :].rearrange("p c (v a) -> p c v a", v=3)

            # ---- scalar bias terms -> cmat [4, 7] --------------------
            tmp28 = cp.tile([H, 28], F32, tag="tmp28")
            nc.vector.tensor_scalar_mul(tmp28[:, 0:4], q4, bk_c)
            nc.vector.tensor_scalar_mul(tmp28[:, 4:8], qwyu[:, 12:16], bk_c)
            nc.vector.tensor_scalar_mul(tmp28[:, 8:12], qwyu[:, 4:8], bk_c)
            nc.vector.tensor_scalar_mul(tmp28[:, 12:16], qwyu[:, 8:12], bk_c)
            nc.vector.tensor_mul(tmp28[:, 16:20], a3q, v4)
            nc.vector.tensor_scalar_mul(tmp28[:, 20:24], v4, b1_c)
            nc.vector.tensor_scalar_mul(tmp28[:, 24:28], q4, tb_c)
            cmat = cp.tile([4, 7], F32, tag="cmat")
            for g in range(7):
                ps_c = sps.tile([4, 1], F32, tag="sps")
                nc.tensor.matmul(ps_c, lhsT=tmp28[:, 4 * g:4 * g + 4],
                                 rhs=ones_col, start=True, stop=True)
                nc.vector.tensor_copy(cmat[:, g:g + 1], ps_c)
            # groups: 0 cbk, 1 cu, 2 cw, 3 cy, 4 c3, 5 c5, 6 c6

            # ---- unified streams: V/ST/SD/SX + early V-product -------
            pp_tiles = {}

            def stream_all(a):
                xa = sp.tile([128, DC, 3, N], BF16, tag="xs")
                nc.sync.dma_start(out=xa, in_=xtd.ap()[a])
                stream_vstsd(a, xa)
                stream_sx(a, xa, g4v)
                vv_a = va_tiles[a]
                pprod = bp.tile([H, N], F32, tag="pp", bufs=4)
                nc.vector.tensor_mul(pprod, vv_a[:, 0:N], vv_a[:, N:2 * N])
                pp_tiles[a] = pprod

            attz_ref = [None]

            def finalpass():
                # ---- row algebra [4, N] (binary-mask collapsed) + finals -
                attz = cp.tile([H, A], F32, tag="attz")
                attz_ref[0] = attz
                zb4 = cp.tile([4, 1], F32, tag="zb4")
                nc.vector.memset(zb4, 0.0)
                st4 = rows5[:, 0:N]
                sd4 = rows5[:, N:2 * N]
                sxq = rows5[:, 2 * N:3 * N]
                sxw = rows5[:, 3 * N:4 * N]
                sxy = rows5[:, 4 * N:5 * N]
                cbk4 = cmat[:, 0:1]

                e_tw = rp.tile([4, N], F32, tag="e_tw")
                nc.vector.scalar_tensor_tensor(e_tw, st4, cbk4, mrep,
                                               op0=OP.add, op1=OP.mult)
                nc.vector.tensor_add(e_tw, e_tw, neg_r)
                z_tw = rp.tile([4, 1], F32, tag="z_tw")
                nc.scalar.activation(e_tw, e_tw, AF.Exp, bias=zb4,
                                     scale=inv_s, accum_out=z_tw)

                e_wi = rp.tile([4, N], F32, tag="e_wi")
                nc.vector.scalar_tensor_tensor(e_wi, sxq, cbk4, mrep,
                                               op0=OP.add, op1=OP.mult)
                nc.vector.tensor_add(e_wi, e_wi, neg_r)
                z_wi = rp.tile([4, 1], F32, tag="z_wi")
                nc.scalar.activation(e_wi, e_wi, AF.Exp, bias=zb4,
                                     scale=inv_s, accum_out=z_wi)

                # fmask is binary -> all DW mask powers collapse to m
                cdw = rp.tile([4, 1], F32, tag="cdw")
                nc.vector.tensor_add(cdw, cmat[:, 1:2], cmat[:, 5:6])
                nc.vector.tensor_add(cdw, cdw, cmat[:, 2:3])
                nc.vector.tensor_add(cdw, cdw, cmat[:, 4:5])
                nc.vector.tensor_add(cdw, cdw, cmat[:, 3:4])
                nc.vector.tensor_add(cdw, cdw, cmat[:, 6:7])
                e_dw = rp.tile([4, N], F32, tag="e_dw")
                nc.vector.tensor_add(e_dw, sd4, sxw)
                nc.vector.tensor_add(e_dw, e_dw, sxy)
                nc.vector.scalar_tensor_tensor(e_dw, e_dw, cdw, mrep,
                                               op0=OP.add, op1=OP.mult)
                nc.vector.tensor_add(e_dw, e_dw, neg_r)
                z_dw = rp.tile([4, 1], F32, tag="z_dw")
                nc.scalar.activation(e_dw, e_dw, AF.Exp, bias=zb4,
                                     scale=inv_s, accum_out=z_dw)

                zmat = rp.tile([4, 3], F32, tag="zmat")
                nc.vector.tensor_copy(zmat[:, 0:1], z_tw)
                nc.vector.tensor_copy(zmat[:, 1:2], z_wi)
                nc.vector.tensor_copy(zmat[:, 2:3], z_dw)
                rz = rp.tile([4, 3], F32, tag="rz")
                nc.vector.reciprocal(rz, zmat)
                alpha = rp.tile([4, 3], F32, tag="alpha")
                nc.vector.tensor_mul(alpha, rz, combw4)

                att = rp.tile([4, N], F32, tag="att")
                nc.vector.tensor_scalar_mul(att, e_tw, alpha[:, 0:1])
                nc.vector.scalar_tensor_tensor(att, e_wi, alpha[:, 1:2], att,
                                               op0=OP.mult, op1=OP.add)
                nc.vector.scalar_tensor_tensor(att, e_dw, alpha[:, 2:3], att,
                                               op0=OP.mult, op1=OP.add)
                rbf4 = rp.tile([4, N], BF16, tag="rbf4")
                nc.vector.tensor_mul(rbf4, att, mrep)

                for a in range(A):
                    ps_rb = sps.tile([H, N], F32, tag="sps")
                    nc.tensor.matmul(
                        ps_rb, lhsT=bpk[0:4, B_E4 + a * H:B_E4 + (a + 1) * H],
                        rhs=rbf4, start=True, stop=True)
                    p2 = bp.tile([H, N], F32, tag="pp2")
                    nc.vector.tensor_tensor(p2, pp_tiles[a], ps_rb, op=OP.mult)
                    nc.vector.reduce_sum(attz[:, a:a + 1], p2,
                                         axis=mybir.AxisListType.X)

            stream_all(0)
            stream_all(1)
            stream_all(2)
            stream_all(3)
            finalpass()
            nc.sync.dma_start(out=out.ap(), in_=attz_ref[0])

    nc.compile()
    return nc


def _prep_inputs(inputs):
    f = {k: np.asarray(v, dtype=np.float32) for k, v in inputs.items()}
    cpack = np.zeros((128, C_W), np.float32)
    cpack[:, C_WKT:C_WKT + D] = f["Wk"].T
    cpack[:, C_WQ:C_WQ + DC * H] = np.transpose(
        f["Wq"].reshape(DC, 128, H), (1, 0, 2)).reshape(128, DC * H)
    cpack[:, C_TAT:C_TAT + H] = f["trans_W"][:H].T
    cpack[:, C_TBT:C_TBT + H] = f["trans_W"][H:].T
    cpack[:, C_W1A:C_W1A + H] = f["W1_W"][:H]
    cpack[:, C_W1BT:C_W1BT + H] = f["W1_W"][H:].T
    for i, k in enumerate(("bq", "bk", "bv", "W1_b", "trans_b")):
        cpack[:, C_BIAS + i] = f[k]
    cpack[0:4, C_COMBW:C_COMBW + 3] = np.tile(f["comb_w"], (4, 1))

    bpack = np.zeros((128, B_W), np.float32)
    bpack[:, B_WV:B_WV + DC * H] = np.transpose(
        f["Wv"].reshape(DC, 128, H), (1, 0, 2)).reshape(128, DC * H)
    for a in range(A):
        bpack[a, B_E4 + a * H:B_E4 + (a + 1) * H] = 1.0
    bpack = bpack.astype(BF)

    shared = {"t1f": f["T1"].reshape(H, H * H).astype(ml_dtypes.float8_e4m3fn), "bpack": bpack}
    in_maps = []
    for b in range(NCORES):
        cp_b = cpack.copy()
        cp_b[:, C_ASP:C_ASP + DC * A] = np.transpose(
            f["aspect_feature"][b].T.reshape(DC, 128, A),
            (1, 0, 2)).reshape(128, DC * A)
        cp_b[0:4, C_MASK:C_MASK + N] = np.tile(f["fmask"][b], (4, 1))
        m = dict(shared)
        m["cpack"] = cp_b
        cph_b = np.zeros((128, 1057), np.float32)
        cph_b[:, 0:DC * H] = cp_b[:, C_WQ:C_WQ + DC * H]
        cph_b[:, DC * H:DC * H + DC * A] = cp_b[:, C_ASP:C_ASP + DC * A]
        cph_b[:, 1056] = f["bq"]
        m["cpackh"] = cph_b
        xs = np.stack([f["feature"][b], f["all_type_feature"][b],
                       f["dep_feature"][b]], axis=2)  # [A, N, 3, D]
        # -> [A, 128(p), DC(c), 3, N]: element (a,p,c,s,n) = xs[a,n,s,c*128+p]
        m["xtd"] = np.ascontiguousarray(
            xs.transpose(0, 3, 2, 1).reshape(A, DC, 128, 3, N)
              .transpose(0, 2, 1, 3, 4)).astype(BF)
        in_maps.append(m)
    return in_maps


def _install_ntff_shim():
    """Provide antenv.axon_hooks (absent in this image) so trace=True can
    drive NTFF capture through libaxon_pjrt.so."""
    if "antenv.axon_hooks" in sys.modules:
        return
    import antenv

    mod = types.ModuleType("antenv.axon_hooks")
    mod._hook = None
    mod.set_axon_ntff_profile_hook = lambda h: setattr(mod, "_hook", h)
    mod.get_axon_ntff_profile_hook = lambda: mod._hook
    sys.modules["antenv.axon_hooks"] = mod
    antenv.axon_hooks = mod

    so_path = "/opt/axon/libaxon_pjrt.so"
    try:
        lib = ctypes.CDLL(so_path)
    except OSError:
        return
    if not hasattr(lib, "axon_start_nrt_profile"):
        return
    lib.axon_start_nrt_profile.argtypes = [ctypes.POINTER(ctypes.c_int64),
                                           ctypes.c_size_t]
    lib.axon_start_nrt_profile.restype = ctypes.c_int64
    lib.axon_stop_nrt_profile.argtypes = [ctypes.c_char_p]
    lib.axon_stop_nrt_profile.restype = ctypes.c_int64

    @contextlib.contextmanager
    def _hook(output_dir, device_ids):
        import jax

        jax.devices()
        if device_ids:
            ids = (ctypes.c_int64 * len(device_ids))(*device_ids)
            rc = lib.axon_start_nrt_profile(ids, len(device_ids))
        else:
            rc = lib.axon_start_nrt_profile(None, 0)
        if rc != 0:
            raise RuntimeError(f"axon_start_nrt_profile rc={rc}")
        try:
            yield
        finally:
            n = lib.axon_stop_nrt_profile(str(output_dir).encode())
            print(f"profile: {n} file(s) written to {output_dir}")

    mod.set_axon_ntff_profile_hook(_hook)


def kernel(feature, dep_feature, aspect_feature, all_type_feature, fmask,
           Wq, bq, Wk, bk, Wv, bv, trans_W, trans_b, T1, W1_W, W1_b, comb_w,
           _profile=False, _tmpdir=None):
    global LAST_RESULTS
    inputs = dict(feature=feature, dep_feature=dep_feature,
                  aspect_feature=aspect_feature,
                  all_type_feature=all_type_feature, fmask=fmask, Wq=Wq,
                  bq=bq, Wk=Wk, bk=bk, Wv=Wv, bv=bv, trans_W=trans_W,
                  trans_b=trans_b, T1=T1, W1_W=W1_W, W1_b=W1_b,
                  comb_w=comb_w)
    nc = _build()
    in_maps = _prep_inputs(inputs)
    if _profile:
        _install_ntff_shim()
    res = run_bass_kernel_spmd(nc, in_maps, list(range(NCORES)),
                               trace=_profile, tmpdir=_tmpdir)
    LAST_RESULTS = res
    full = np.stack([res.results[c]["out"].T for c in range(NCORES)])
    return full.astype(np.float32)

